# revision 29
# baseline (speedup 1.0000x reference)
"""Trainium2 Bass kernel for nn_C3DNet — data-parallel over the 10 samples on 8 cores.

Math (per sample, from the reference):
  x:(52,7,24) -conv1(6,2,2)s(2,1,2)+sig-> (24,6,12) -conv2(4,1,2)s(4,1,2)+sig-> (6,6,6)
  -avgpool2-> 27 -fc4+sig-> 80 -fc5+sig-> 200 -fc6+sig-> 676
  out = h6.reshape(13,52) @ x.reshape(52,168)  -> (13,168) -> 2184

Everything is cast as TensorE matmuls (f32 PSUM). Numerics:
  * conv1 and fc6 run in fp8e4m3 DoubleRow mode: the contraction dim is folded
    in half onto the partitions with a k-tile dim of 2, so each matmul streams
    half the stationary rows and fc6 needs 13 matmuls instead of 26.
  * fc5's sigmoids are emitted as tanh(x/2) = 2*sigmoid(x)-1 (same ACT table
    set as sigmoid, so no extra table load). Centering the fc6 moving operand
    around 0 halves its fp8 quantization error; w6/2 and the folded bias
    b6 + rowsum(w6)/2 are baked into the fp8 stationary host-side. The folded
    bias rides the two k-tile ones-rows as a hi+residual fp8 pair (33x more
    accurate than a single fp8 row). Same split-bias trick for conv1's b1.
  * conv2/fc4/fc5/einsum stay bf16; output is stored bf16 and widened on host.
    Emulated end-to-end rel err 4.7e-3 vs the 2e-2 gate (bf16 baseline 2.9e-3).

Schedule notes (from perfetto traces of prior revisions):
  * The measured exec window = [first named instruction, end of the NEFF
    postamble]; the postamble (~254-semaphore file reset split across
    engines, ~5.9us serial per engine after an all-engine rendezvous) is
    fixed runtime cost. A trivial 2-DMA kernel measures ~11.4us traced.
  * HWDGE trigger->16th-completion-credit is ~1.8us; the critical a8 pack
    (conv1 weights + tap-expanded x in ONE fp8 tensor) rides the SP ring
    first so conv1 starts ~2.0us after the window opens.
  * DMA descriptor distribution (measured): a transfer goes wide (~9-16
    engines, >100GB/s) only when it is contiguous, <=~48KB total, and
    <=~64 descriptors; anything bigger/finer pins to a 1-2 engine crawl
    (~20GB/s). The ring also executes DMA instructions SERIALLY (next
    instruction's descriptors flow only after the previous transfer
    drains), and each trigger costs ~600-800ns of sequencer pacing, so
    w68 (fc6's 168KB) ships as four <=26-row pieces on the SP ring,
    ordered exactly by consumption time: a8, w68 x4, wx (einsum weeks,
    needed last), out-store. The Pool SWDGE carries the rest (wb2, w4p,
    w5t) with its own semaphores (HWDGE and SWDGE completion updates must
    NOT share one).
  * The sigmoid ACT table load (~1.3us) is emitted by walrus before the
    first table-using ACT instruction of the bb with NO wait attached, so
    a dep-light dummy sigmoid leads the ACT block and the table streams in
    parallel with the input-DMA wait. Scalar triggers no DMAs (a Scalar
    DMA costs ~1.4us of SEQ occupancy and delays the table).
  * A K=1 f32 warm-up matmul leads the PE stream (no deps): the PE p-state
    ramp (0.65 -> 1.2GHz after 100ns busy) then applies to conv1.
  * Stage gates ride as ATTACHED waits on the first instruction of each
    stage; early-satisfied DMA gates sit as standalone waits off the hot
    handoffs. (h,w)-pooling is one DVE tensor_reduce over the (dh,dw)
    dims. PSUM->SBUF output copies stay on ACT: a DVE tensor_copy from
    PSUM hung the device.
  * Output: ACT copies both einsum regions PSUM->SBUF as bf16 (pipelined
    with the einsum matmuls), SP does one store with the asem gate
    attached; host widens to f32.
  * The bass Block exit barrier (end-block EVENT_SEMAPHOREs) is stripped:
    the runtime postamble begins with its own all-engine rendezvous. The
    per-engine Drains are kept so the output DMA quiesces before the
    postamble's semaphore-file reset.

Raw-bass (Block + explicit semaphores): this walrus build only supports ONE
attached sync-wait per Matmult/DMA instruction, so standalone wait_ge
instructions are used. DMA completion order is not guaranteed across queues,
so consumers wait for the issuing group's FULL credit count (16 per DMA).
"""

import sys
from contextlib import ExitStack

sys.path.insert(0, "/opt/trn_rl_repo")

import numpy as np
import ml_dtypes

_DMA_CREDITS = 16

BF16 = ml_dtypes.bfloat16
FP8 = ml_dtypes.float8_e4m3fn

N_CORES = 8
NS = 2  # sample slots per core
ASSIGN = [[0, 8], [1, 9]] + [[i, i] for i in range(2, N_CORES)]

LAST_EXEC_NS = None
LAST_RESULT = None

_BUILT = {}


def _build_nc():
    import concourse.bass as bass
    import concourse.mybir as mybir

    f32 = mybir.dt.float32
    bf16 = mybir.dt.bfloat16
    fp8 = mybir.dt.float8e4
    Sig = mybir.ActivationFunctionType.Sigmoid
    Tanh = mybir.ActivationFunctionType.Tanh
    DR = mybir.MatmulPerfMode.DoubleRow

    nc = bass.Bass()

    # a8: conv1 fp8 DoubleRow pack. cols 0:256 stationary (tap j: j*64 + t*32
    # + d, d padded 24->32: DoubleRow needs M % 32 == 0), cols 256:1408 moving
    # (256 + j*288 + t*144 + s*72 + h*12 + w).
    # Contraction c = t*27+p: c<52 x-rows, c=52/53 the split-bias ones rows.
    a8_d = nc.declare_dram_parameter("a8", [27, 1408], fp8, isOutput=False)
    # wb2: conv2 stationary (rows 0:25 incl. the b2 ones-row) — tiny and
    # needed early. wx: the einsum weeks, only needed ~5us in, rides the
    # Sync ring AFTER the w68 pieces (the ring processes DMA instructions
    # serially, so early-ring bytes directly delay fc6's weights)
    wb2_d = nc.declare_dram_parameter("wb2", [25, 12], bf16, isOutput=False)
    wx_d = nc.declare_dram_parameter("wx", [52, 336], bf16, isOutput=False)
    # w4p row 6 = b4 in the j=0 block, zeros elsewhere
    w4p_d = nc.declare_dram_parameter("w4p", [12, 720], bf16, isOutput=False)
    w5t_d = nc.declare_dram_parameter("w5t", [86, 200], bf16, isOutput=False)
    # w68: fc6 fp8 DoubleRow pack [101, (i, t, jj)] with jj padded 52->64
    # (M % 32 == 0): rows 0:100 = (w6/2).T halves, row 100 = split folded
    # bias (t=0 hi, t=1 residual)
    w68_d = nc.declare_dram_parameter("w68", [101, 1664], fp8, isOutput=False)
    out_d = nc.declare_dram_parameter("out", [13, NS * 168], bf16, isOutput=True)

    es = ExitStack()

    def sb(name, shape, dt=bf16):
        return es.enter_context(nc.sbuf_tensor(name, shape, dt))

    def pt(name, shape):
        return es.enter_context(nc.psum_tensor(name, shape, f32))

    with es:
        a8_t = sb("a8_t", [27, 1408], fp8)
        wb2_t = sb("wb2_t", [25, 12])
        wx_t = sb("wx_t", [52, 336])
        w4p_t = sb("w4p_t", [12, 720])
        w5t_t = sb("w5t_t", [86, 200])
        w68_t = sb("w68_t", [101, 1664], fp8)
        h1_t = sb("h1_t", [25, NS * 72])   # row 24 = ones (b2 rides wb2 row 24)
        h2_t = sb("h2_t", [6, NS * 36])
        pool_t = sb("pool_t", [7, NS * 9])  # row 6 = ones (b4 rides w4p row 6)
        h4_t = sb("h4_t", [81, NS])         # row 80 = ones (b5 rides w5t row 80)
        t01 = sb("t01", [101, 2 * NS], fp8)  # tanh halves; row 100 = ones
        h6_t = sb("h6_t", [52, 13 * NS])
        out_t = sb("out_t", [13, NS * 168])  # bf16 output staging
        scr_t = sb("scr_t", [1, 2])          # dummy-act output (table trigger)
        zb_t = sb("zb_t", [101, 2], f32)     # zero bias + dummy operands

        psum1 = pt("psum1", [32, NS * 72])  # rows 24:32 = DoubleRow M-pad junk
        psum2 = pt("psum2", [6, NS * 36])
        psum4 = pt("psum4", [80, NS])
        psum5 = pt("psum5", [100, 2 * NS])
        psum6 = pt("psum6", [64, 13 * NS])  # rows 52:64 = DoubleRow M-pad junk
        psume = pt("psume", [13, NS * 168])
        psum_w = pt("psum_w", [1, 2])        # PE warm-up target

        dsA = es.enter_context(nc.semaphore("dsA"))    # a8 (SP)
        dsB = es.enter_context(nc.semaphore("dsB"))    # wb2 (SP)
        dsX = es.enter_context(nc.semaphore("dsX"))    # wx weeks (SP)
        dsE = es.enter_context(nc.semaphore("dsE"))    # w4p (SWDGE)
        dsF = es.enter_context(nc.semaphore("dsF"))    # w5t (SWDGE)
        dsG = es.enter_context(nc.semaphore("dsG"))    # w68 row pieces x4 (SP)
        dsO = es.enter_context(nc.semaphore("dsO"))    # output store (no waiter)
        ssem = es.enter_context(nc.semaphore("ssem"))  # Pool memsets (target 5)
        psem = es.enter_context(nc.semaphore("psem"))
        asem = es.enter_context(nc.semaphore("asem"))
        vsem = es.enter_context(nc.semaphore("vsem"))

        with nc.Block() as block:
            hoist = nc._hoist_insts = []

            @block.sync
            def _(sync):
                # a8 first (conv1's gate), then the tiny conv2 stationary,
                # then w5t, then the three w68 pieces, then the einsum weeks
                # (needed last). The ring executes DMA instructions serially,
                # so the order is exactly the consumption order.
                hoist.append(sync.dma_start(out=a8_t[:], in_=a8_d[:]).then_inc(dsA, 16))
                for r0, r1 in ((0, 26), (26, 52), (52, 77), (77, 101)):
                    hoist.append(
                        sync.dma_start(
                            out=w68_t[r0:r1, :], in_=w68_d[r0:r1, :]
                        ).then_inc(dsG, 16)
                    )
                hoist.append(sync.dma_start(out=wx_t[:], in_=wx_d[:]).then_inc(dsX, 16))
                # single output store once both ACT copies land (wait attached
                # to the trigger)
                sync.dma_start(out=out_d[:, :], in_=out_t[:])._wait_ge(asem, 7).then_inc(dsO, 16)

            @block.vector
            def _(vector):
                # (h, w) pooling as ONE 4-tap reduce over the (dh, dw) dims
                h2r = h2_t[:].rearrange(
                    "p (s hp dh wp dw) -> p (s hp) wp dh dw", s=NS, hp=3, dh=2, wp=3, dw=2
                )
                poolr = pool_t[0:6, :].rearrange("p (s hp wp) -> p (s hp) wp", s=NS, hp=3, wp=3)
                with nc.allow_low_precision("4-term bf16 pooling sum, matches prior impl"):
                    vector.tensor_reduce(
                        poolr[:], h2r[:], axis=mybir.AxisListType.XY, op=mybir.AluOpType.add
                    )._wait_ge(asem, 2).then_inc(vsem)  # 1


            @block.gpsimd
            def _(gpsimd):
                # w4p first on the Pool SWDGE (fc4 needs it earliest of the
                # SWDGE loads), then the memsets (the ACT dummy waits
                # ssem>=5), then the third w68 piece (moving it off the
                # 6-DMA Sync ring whose congested completion path stalled
                # fc6 ~1us). HWDGE and SWDGE must not share a semaphore.
                # Whole-tensor memsets: APs must start at partition 0.
                hoist.append(gpsimd.dma_start(out=wb2_t[:], in_=wb2_d[:]).then_inc(dsB, 16))
                hoist.append(gpsimd.dma_start(out=w4p_t[:], in_=w4p_d[:]).then_inc(dsE, 16))
                hoist.append(gpsimd.memset(h1_t[:], 1.0).then_inc(ssem))
                hoist.append(gpsimd.memset(pool_t[:], 1.0).then_inc(ssem))
                hoist.append(gpsimd.memset(h4_t[:], 1.0).then_inc(ssem))
                hoist.append(gpsimd.memset(t01[:], 1.0).then_inc(ssem))
                hoist.append(gpsimd.memset(zb_t[:], 0.0).then_inc(ssem))
                hoist.append(gpsimd.dma_start(out=w5t_t[:], in_=w5t_d[:]).then_inc(dsF, 16))

            @block.scalar
            def _(scalar):
                # Scalar stays DMA-free so its sequencer reaches the table
                # load ASAP. dummy sigmoid FIRST IN THE BB: walrus emits the
                # ACT_TABLE_LOAD right before the first table-using ACTIVATE
                # of the bb with no wait attached, so the ~1.3us table stream
                # overlaps the input-DMA completion wait. The dummy's own wait
                # covers the memsets.
                scalar.activation(scr_t[:], zb_t[0:1, 0:2], Sig, bias=zb_t[0:1, 0:1])._wait_ge(ssem, 5)
                scalar.activation(h1_t[0:24, :], psum1[0:24, :], Sig, bias=zb_t[0:24, 0:1])._wait_ge(psem, 1).then_inc(asem)  # 1
                scalar.activation(h2_t[:], psum2[:], Sig, bias=zb_t[0:6, 0:1])._wait_ge(psem, 2).then_inc(asem)  # 2
                scalar.activation(h4_t[0:80, :], psum4[:], Sig, bias=zb_t[0:80, 0:1])._wait_ge(psem, 3).then_inc(asem)  # 3
                # fc5 emits tanh(x/2) = 2*sigmoid(x)-1 in fp8 (fc6's
                # DoubleRow moving operand needs BOTH halves, so one ACT op)
                scalar.activation(
                    t01[0:100, :], psum5[:], Tanh, bias=zb_t[0:100, 0:1], scale=0.5
                )._wait_ge(psem, 4).then_inc(asem)  # 4
                scalar.activation(h6_t[:], psum6[0:52, :], Sig, bias=zb_t[0:52, 0:1])._wait_ge(psem, 5).then_inc(asem)  # 5
                scalar.copy(out_t[:, 0:168], psume[:, 0:168])._wait_ge(psem, 6).then_inc(asem)
                scalar.copy(out_t[:, 168:336], psume[:, 168:336])._wait_ge(psem, 7).then_inc(asem)

            @block.tensor
            def _(tensor):
                # p-state warm-up: a dep-free K=1 f32 matmul so the PE clock
                # ramps before conv1 (reads zb_t racily; output unused)
                tensor.matmul(psum_w[:], zb_t[0:1, 0:1], zb_t[0:1, 0:2], start=True, stop=True)
                # conv1: 4 fp8 DoubleRow matmuls, contraction 2x27 (52 x-rows
                # + split-b1 ones rows). The a8 gate rides the first matmul.
                a8w = a8_t[:, 0:256].rearrange("p (j t d) -> p j t d", j=4, t=2, d=32)
                a8x = a8_t[:, 256:1408].rearrange("p (j t n) -> p j t n", j=4, t=2, n=144)
                for j in range(4):
                    mm = tensor.matmul(
                        psum1[:],
                        a8w[:, j],
                        a8x[:, j],
                        start=(j == 0),
                        stop=(j == 3),
                        perf_mode=DR,
                    )
                    if j == 0:
                        mm._wait_ge(dsA, _DMA_CREDITS)
                    if j == 3:
                        mm.then_inc(psem)  # psem 1
                # conv2: K=25 incl. the b2 ones-row; stationary lives in b16
                tensor.wait_ge(dsB, _DMA_CREDITS)
                h14 = h1_t[:].rearrange("p (s h w) -> p s h w", s=NS, h=6, w=12)
                for kw in range(2):
                    mm = tensor.matmul(
                        psum2[:],
                        wb2_t[0:25, kw * 6 : (kw + 1) * 6],
                        h14[:, :, :, kw : kw + 11 : 2],
                        start=(kw == 0),
                        stop=(kw == 1),
                    )
                    if kw == 0:
                        mm._wait_ge(asem, 1)
                    if kw == 1:
                        mm.then_inc(psem)  # psem 2
                tensor.wait_ge(dsE, _DMA_CREDITS)
                # fc4: 9 (hp,wp) matmuls vs the h/w-pooled tile; d-pooling and
                # /8 live in w4p; j=0 has K=7 incl. the b4 ones-row
                pool4 = pool_t[:].rearrange("p (s j) -> p s j", s=NS, j=9)
                for j in range(9):
                    kk = 7 if j == 0 else 6
                    mm = tensor.matmul(
                        psum4[:],
                        w4p_t[0:kk, j * 80 : (j + 1) * 80],
                        pool4[0:kk, :, j],
                        start=(j == 0),
                        stop=(j == 8),
                    )
                    if j == 0:
                        mm._wait_ge(vsem, 1)
                    if j == 8:
                        mm.then_inc(psem)  # psem 3
                # fc5: two k-halves into one psum (one tanh covers both)
                tensor.wait_ge(dsF, _DMA_CREDITS)
                tensor.matmul(
                    psum5[:, 0:NS], w5t_t[0:81, 0:100], h4_t[:], start=True, stop=True
                )._wait_ge(asem, 3)
                tensor.matmul(
                    psum5[:, NS : 2 * NS], w5t_t[0:81, 100:200], h4_t[:], start=True, stop=True
                ).then_inc(psem)  # psem 4
                # fc6: 13 fp8 DoubleRow matmuls, contraction 2x101 (200 tanh
                # rows + split-b6c ones rows); needs BOTH fc5 halves (asem>=5)
                tensor.wait_ge(dsG, 4 * _DMA_CREDITS)
                w68v = w68_t[:].rearrange("p (i t jj) -> p i t jj", i=13, t=2, jj=64)
                t01v = t01[:].rearrange("p (t s) -> p t s", t=2, s=NS)
                for i in range(13):
                    mm = tensor.matmul(
                        psum6[:, i * NS : (i + 1) * NS],
                        w68v[:, i],
                        t01v[:],
                        start=True,
                        stop=True,
                        perf_mode=DR,
                    )
                    if i == 0:
                        mm._wait_ge(asem, 4)
                    if i == 12:
                        mm.then_inc(psem)  # psem 5
                # einsum; each sample region releases its own copy engine
                tensor.wait_ge(dsX, _DMA_CREDITS)
                h6v = h6_t[:].rearrange("p (i s) -> p s i", s=NS)
                for s in range(NS):
                    mm = tensor.matmul(
                        psume[:, s * 168 : (s + 1) * 168],
                        h6v[:, s, :],
                        wx_t[0:52, s * 168 : (s + 1) * 168],
                        start=True,
                        stop=True,
                    ).then_inc(psem)  # psem 6, 7
                    if s == 0:
                        mm._wait_ge(asem, 5)

    _strip_barriers(nc)
    return nc


def _strip_barriers(nc):
    f = nc.m.functions[0]
    bbs = {bb.name: bb for bb in f.blocks}
    main = bbs["main"]
    # 1) drop the init all-engine barrier (nothing reads the const-AP tiles)
    main.instructions = [
        i
        for i in main.instructions
        if not (
            i.name.startswith("barrier_")
            or getattr(i, "opcode", "") == "Drain"
            or type(i).__name__ == "InstDrain"
        )
    ]
    # 2) drop the Block exit barrier (the runtime postamble begins with its
    #    own all-engine rendezvous); keep the Drains so the output DMA
    #    quiesces before the postamble's semaphore-file reset
    for bb in f.blocks:
        if bb.name.endswith("_end"):
            bb.instructions = [
                i for i in bb.instructions if type(i).__name__ != "InstEventSemaphore"
            ]
    # 3) hoist the input-DMA triggers + memsets into main so they start during
    #    the preamble, before the Block-entry rendezvous
    hoisted = {bi.ins.name for bi in getattr(nc, "_hoist_insts", [])}
    if hoisted:
        moved = []
        for bb in f.blocks:
            if bb.name == "main" or not bb.instructions:
                continue
            keep = []
            for i in bb.instructions:
                (moved if i.name in hoisted else keep).append(i)
            if len(keep) != len(bb.instructions):
                bb.instructions = keep
        # insert at the very top of main (after the entry Call): the DMA
        # triggers use only immediates + the parameter table, not the
        # preamble registers
        insts = main.instructions
        main.instructions = insts[:1] + moved + insts[1:]


def _prep_weights(w1, b1, w2, b2, w4, b4, w5, b5, w6, b6):
    f = np.float32
    w1v = np.asarray(w1, f)[0, 0]  # (6,2,2)
    w2v = np.asarray(w2, f)[0, 0, :, 0, :]  # (4,2)
    w4 = np.asarray(w4, f)
    w5 = np.asarray(w5, f)
    w6 = np.asarray(w6, f)
    b1 = np.asarray(b1, f)
    b2 = np.asarray(b2, f)
    b4 = np.asarray(b4, f)
    b5 = np.asarray(b5, f)
    b6 = np.asarray(b6, f)

    # conv1 stationary, fp8 DoubleRow: WW[c, j, d] over contraction c;
    # d padded 24->32 (DoubleRow M % 32 == 0), pad cols stay zero
    WW = np.zeros((54, 4, 32), f)
    for kd in range(6):
        for kh in range(2):
            for kw in range(2):
                for d in range(24):
                    WW[2 * d + kd, kh * 2 + kw, d] = w1v[kd, kh, kw]
    u1 = np.asarray(b1[0], FP8).astype(f)
    WW[52, 0, 0:24] = u1            # bias hi, tap 0 only
    WW[53, 0, 0:24] = b1[0] - u1    # bias residual (fp8 of the remainder)
    wb8 = np.transpose(WW.reshape(2, 27, 4, 32), (1, 2, 0, 3))  # (p, j, t, d)

    # conv2 stationary block (bf16)
    wb2 = np.zeros((25, 12), f)
    for kd in range(4):
        for kw in range(2):
            for d in range(6):
                wb2[4 * d + kd, kw * 6 + d] = w2v[kd, kw]
    wb2[24, 0:6] = b2[0]  # ones-row bias, kw=0 block only

    w4r = w4.reshape(80, 3, 3, 3) / 8.0
    w4q = np.transpose(w4r, (1, 2, 3, 0)).reshape(3, 720)
    w4p = np.zeros((12, 720), f)
    w4p[0:6:2, :] = w4q
    w4p[1:6:2, :] = w4q
    w4p[6, 0:80] = b4  # ones-row bias, j=0 block only

    w5t = np.zeros((86, 200), f)
    w5t[0:80, :] = w5.T
    w5t[80, :] = b5

    # fc6 stationary, fp8 DoubleRow over tanh-centered fc5 outputs:
    # h6pre = (w6/2) @ tanh + (b6 + rowsum(w6)/2); jj padded 52->64
    W6E = np.zeros((202, 13, 64), f)
    w6h = 0.5 * w6.T.reshape(200, 13, 52)  # (k, i, jj)
    W6E[0:100, :, 0:52] = w6h[0:100]
    W6E[101:201, :, 0:52] = w6h[100:200]
    b6c = b6 + 0.5 * w6.sum(axis=1)
    u6 = np.asarray(b6c, FP8).astype(f)
    W6E[100, :, 0:52] = u6.reshape(13, 52)                # (p=100, t=0): bias hi
    W6E[201, :, 0:52] = (b6c - u6).reshape(13, 52)        # (p=100, t=1): residual
    # (p, t) -> c = t*101 + p; cols = i*128 + t*64 + jj
    w68 = np.transpose(W6E.reshape(2, 101, 13, 64), (1, 2, 0, 3)).reshape(101, 1664)

    return dict(
        w4p=np.ascontiguousarray(w4p.astype(BF16)),
        w5t=np.ascontiguousarray(w5t.astype(BF16)),
        w68=np.ascontiguousarray(w68.astype(FP8)),
    ), wb8, wb2


def kernel(x, w1, b1, w2, b2, w4, b4, w5, b5, w6, b6, _trace=False):
    global LAST_EXEC_NS, LAST_RESULT
    from concourse.bass_utils import run_bass_kernel_spmd

    if "nc" not in _BUILT:
        _BUILT["nc"] = _build_nc()
    nc = _BUILT["nc"]

    xs = np.ascontiguousarray(np.asarray(x, np.float32).reshape(10, 52, 168))
    wd, wb8, wb2 = _prep_weights(w1, b1, w2, b2, w4, b4, w5, b5, w6, b6)

    in_maps = []
    for i in range(N_CORES):
        xc = np.transpose(np.stack([xs[a] for a in ASSIGN[i]]), (1, 0, 2))  # (52, NS, 168)


        # a8: conv1 fp8 pack. XS[c, j, s, h, w] over contraction c (54).
        x4 = xc.reshape(52, NS, 7, 24)
        XS = np.ones((54, 4, NS, 6, 12), np.float32)
        for kh in range(2):
            for kw in range(2):
                XS[0:52, kh * 2 + kw] = x4[:, :, kh : kh + 6, kw : kw + 23 : 2]
        x8 = np.transpose(XS.reshape(2, 27, 4, NS * 72), (1, 2, 0, 3))  # (p, j, t, n)

        ac = np.zeros((27, 1408), np.float32)
        ac[:, 0:256] = wb8.reshape(27, 256)
        ac[:, 256:1408] = x8.reshape(27, 1152)

        m = {
            "a8": np.ascontiguousarray(ac.astype(FP8)),
            "wb2": np.ascontiguousarray(wb2.astype(BF16)),
            "wx": np.ascontiguousarray(xc.reshape(52, NS * 168).astype(BF16)),
        }
        m.update(wd)
        in_maps.append(m)

    res = run_bass_kernel_spmd(nc, in_maps, core_ids=list(range(N_CORES)), trace=_trace)
    LAST_EXEC_NS = res.exec_time_ns
    LAST_RESULT = res

    out = np.zeros((10, 2184), np.float32)
    for i in range(N_CORES):
        o = res.results[i]["out"].astype(np.float32).reshape(13, NS, 168)
        for slot, b in enumerate(ASSIGN[i]):
            out[b] = o[:, slot, :].reshape(2184)
    return out


# revision 30
# speedup vs baseline: 1.0065x; 1.0065x over previous
"""Trainium2 Bass kernel for nn_C3DNet — data-parallel over the 10 samples on 8 cores.

Math (per sample, from the reference):
  x:(52,7,24) -conv1(6,2,2)s(2,1,2)+sig-> (24,6,12) -conv2(4,1,2)s(4,1,2)+sig-> (6,6,6)
  -avgpool2-> 27 -fc4+sig-> 80 -fc5+sig-> 200 -fc6+sig-> 676
  out = h6.reshape(13,52) @ x.reshape(52,168)  -> (13,168) -> 2184

Everything is cast as TensorE matmuls (f32 PSUM). Numerics:
  * conv1 and fc6 run in fp8e4m3 DoubleRow mode: the contraction dim is folded
    in half onto the partitions with a k-tile dim of 2, so each matmul streams
    half the stationary rows and fc6 needs 13 matmuls instead of 26.
  * fc5's sigmoids are emitted as tanh(x/2) = 2*sigmoid(x)-1 (same ACT table
    set as sigmoid, so no extra table load). Centering the fc6 moving operand
    around 0 halves its fp8 quantization error; w6/2 and the folded bias
    b6 + rowsum(w6)/2 are baked into the fp8 stationary host-side. The folded
    bias rides the two k-tile ones-rows as a hi+residual fp8 pair (33x more
    accurate than a single fp8 row). Same split-bias trick for conv1's b1.
  * conv2/fc4/fc5/einsum stay bf16; output is stored bf16 and widened on host.
    Emulated end-to-end rel err 4.7e-3 vs the 2e-2 gate (bf16 baseline 2.9e-3).

Schedule notes (from perfetto traces of prior revisions):
  * The measured exec window = [first named instruction, end of the NEFF
    postamble]; the postamble (~254-semaphore file reset split across
    engines, ~5.9us serial per engine after an all-engine rendezvous) is
    fixed runtime cost. A trivial 2-DMA kernel measures ~11.4us traced.
  * HWDGE trigger->16th-completion-credit is ~1.8us; the critical a8 pack
    (conv1 weights + tap-expanded x in ONE fp8 tensor) rides the SP ring
    first so conv1 starts ~2.0us after the window opens.
  * DMA descriptor distribution (measured): a transfer goes wide (~9-16
    engines, >100GB/s) only when it is contiguous, <=~48KB total, and
    <=~64 descriptors; anything bigger/finer pins to a 1-2 engine crawl
    (~20GB/s). The ring also executes DMA instructions SERIALLY (next
    instruction's descriptors flow only after the previous transfer
    drains), and each trigger costs ~600-800ns of sequencer pacing, so
    w68 (fc6's 168KB) ships as four <=26-row pieces on the SP ring,
    ordered exactly by consumption time: a8, w68 x4, wx (einsum weeks,
    needed last), out-store. The Pool SWDGE carries the rest (wb2, w4p,
    w5t) with its own semaphores (HWDGE and SWDGE completion updates must
    NOT share one).
  * The sigmoid ACT table load (~1.3us) is emitted by walrus before the
    first table-using ACT instruction of the bb with NO wait attached, so
    a dep-light dummy sigmoid leads the ACT block and the table streams in
    parallel with the input-DMA wait. Scalar triggers no DMAs (a Scalar
    DMA costs ~1.4us of SEQ occupancy and delays the table).
  * A K=1 f32 warm-up matmul leads the PE stream (no deps): the PE p-state
    ramp (0.65 -> 1.2GHz after 100ns busy) then applies to conv1.
  * Stage gates ride as ATTACHED waits on the first instruction of each
    stage; early-satisfied DMA gates sit as standalone waits off the hot
    handoffs. (h,w)-pooling is one DVE tensor_reduce over the (dh,dw)
    dims. PSUM->SBUF output copies stay on ACT: a DVE tensor_copy from
    PSUM hung the device.
  * Output: ACT copies both einsum regions PSUM->SBUF as bf16 (pipelined
    with the einsum matmuls), SP does one store with the asem gate
    attached; host widens to f32.
  * The bass Block exit barrier (end-block EVENT_SEMAPHOREs) is stripped:
    the runtime postamble begins with its own all-engine rendezvous. The
    per-engine Drains are kept so the output DMA quiesces before the
    postamble's semaphore-file reset.

Raw-bass (Block + explicit semaphores): this walrus build only supports ONE
attached sync-wait per Matmult/DMA instruction, so standalone wait_ge
instructions are used. DMA completion order is not guaranteed across queues,
so consumers wait for the issuing group's FULL credit count (16 per DMA).
"""

import sys
from contextlib import ExitStack

sys.path.insert(0, "/opt/trn_rl_repo")

import numpy as np
import ml_dtypes

_DMA_CREDITS = 16

BF16 = ml_dtypes.bfloat16
FP8 = ml_dtypes.float8_e4m3fn

N_CORES = 8
NS = 2  # sample slots per core
ASSIGN = [[0, 8], [1, 9]] + [[i, i] for i in range(2, N_CORES)]

LAST_EXEC_NS = None
LAST_RESULT = None

_BUILT = {}


def _build_nc():
    import concourse.bass as bass
    import concourse.mybir as mybir

    f32 = mybir.dt.float32
    bf16 = mybir.dt.bfloat16
    fp8 = mybir.dt.float8e4
    Sig = mybir.ActivationFunctionType.Sigmoid
    Tanh = mybir.ActivationFunctionType.Tanh
    DR = mybir.MatmulPerfMode.DoubleRow

    nc = bass.Bass()

    # a8: conv1 fp8 DoubleRow pack. cols 0:256 stationary (tap j: j*64 + t*32
    # + d, d padded 24->32: DoubleRow needs M % 32 == 0), cols 256:1408 moving
    # (256 + j*288 + t*144 + s*72 + h*12 + w).
    # Contraction c = t*27+p: c<52 x-rows, c=52/53 the split-bias ones rows.
    a8_d = nc.declare_dram_parameter("a8", [27, 1408], fp8, isOutput=False)
    # wb2: conv2 stationary (rows 0:25 incl. the b2 ones-row) — tiny and
    # needed early. wx: the einsum weeks, only needed ~5us in, rides the
    # Sync ring AFTER the w68 pieces (the ring processes DMA instructions
    # serially, so early-ring bytes directly delay fc6's weights)
    wb2_d = nc.declare_dram_parameter("wb2", [25, 12], bf16, isOutput=False)
    wx_d = nc.declare_dram_parameter("wx", [52, 336], bf16, isOutput=False)
    # w4p row 6 = b4 in the j=0 block, zeros elsewhere
    w4p_d = nc.declare_dram_parameter("w4p", [12, 720], bf16, isOutput=False)
    w5t_d = nc.declare_dram_parameter("w5t", [86, 200], bf16, isOutput=False)
    # w68: fc6 fp8 DoubleRow pack [101, (i, t, jj)] with jj padded 52->64
    # (M % 32 == 0): rows 0:100 = (w6/2).T halves, row 100 = split folded
    # bias (t=0 hi, t=1 residual)
    w68_d = nc.declare_dram_parameter("w68", [101, 1664], fp8, isOutput=False)
    out_d = nc.declare_dram_parameter("out", [13, NS * 168], bf16, isOutput=True)

    es = ExitStack()

    def sb(name, shape, dt=bf16):
        return es.enter_context(nc.sbuf_tensor(name, shape, dt))

    def pt(name, shape):
        return es.enter_context(nc.psum_tensor(name, shape, f32))

    with es:
        a8_t = sb("a8_t", [27, 1408], fp8)
        wb2_t = sb("wb2_t", [25, 12])
        wx_t = sb("wx_t", [52, 336])
        w4p_t = sb("w4p_t", [12, 720])
        w5t_t = sb("w5t_t", [86, 200])
        w68_t = sb("w68_t", [101, 1664], fp8)
        h1_t = sb("h1_t", [25, NS * 72])   # row 24 = ones (b2 rides wb2 row 24)
        h2_t = sb("h2_t", [6, NS * 36])
        pool_t = sb("pool_t", [7, NS * 9])  # row 6 = ones (b4 rides w4p row 6)
        h4_t = sb("h4_t", [81, NS])         # row 80 = ones (b5 rides w5t row 80)
        t01 = sb("t01", [101, 2 * NS], fp8)  # tanh halves; row 100 = ones
        h6_t = sb("h6_t", [52, 13 * NS])
        out_t = sb("out_t", [13, NS * 168])  # bf16 output staging
        scr_t = sb("scr_t", [1, 2])          # dummy-act output (table trigger)
        zb_t = sb("zb_t", [101, 2], f32)     # zero bias + dummy operands

        psum1 = pt("psum1", [32, NS * 72])  # rows 24:32 = DoubleRow M-pad junk
        psum2 = pt("psum2", [6, NS * 36])
        psum4 = pt("psum4", [80, NS])
        psum5 = pt("psum5", [100, 2 * NS])
        psum6 = pt("psum6", [64, 13 * NS])  # rows 52:64 = DoubleRow M-pad junk
        psume = pt("psume", [13, NS * 168])
        psum_w = pt("psum_w", [1, 2])        # PE warm-up target

        dsA = es.enter_context(nc.semaphore("dsA"))    # a8 (SP)
        dsB = es.enter_context(nc.semaphore("dsB"))    # wb2 (SP)
        dsX = es.enter_context(nc.semaphore("dsX"))    # wx weeks (SP)
        dsE = es.enter_context(nc.semaphore("dsE"))    # w4p (SWDGE)
        dsF = es.enter_context(nc.semaphore("dsF"))    # w5t (SWDGE)
        dsG = es.enter_context(nc.semaphore("dsG"))    # w68 row pieces x4 (SP)
        dsO = es.enter_context(nc.semaphore("dsO"))    # output store (no waiter)
        ssem = es.enter_context(nc.semaphore("ssem"))  # Pool memsets (target 5)
        psem = es.enter_context(nc.semaphore("psem"))
        asem = es.enter_context(nc.semaphore("asem"))
        vsem = es.enter_context(nc.semaphore("vsem"))

        with nc.Block() as block:
            hoist = nc._hoist_insts = []

            @block.sync
            def _(sync):
                # a8 first (conv1's gate), then the tiny conv2 stationary,
                # then w5t, then the three w68 pieces, then the einsum weeks
                # (needed last). The ring executes DMA instructions serially,
                # so the order is exactly the consumption order.
                hoist.append(sync.dma_start(out=a8_t[:], in_=a8_d[:]).then_inc(dsA, 16))
                for r0, r1 in ((0, 26), (26, 52), (52, 77), (77, 101)):
                    hoist.append(
                        sync.dma_start(
                            out=w68_t[r0:r1, :], in_=w68_d[r0:r1, :]
                        ).then_inc(dsG, 16)
                    )
                hoist.append(sync.dma_start(out=wx_t[:], in_=wx_d[:]).then_inc(dsX, 16))
                # single output store once both ACT copies land (wait attached
                # to the trigger)
                sync.dma_start(out=out_d[:, :], in_=out_t[:])._wait_ge(asem, 7).then_inc(dsO, 16)

            @block.vector
            def _(vector):
                # (h, w) pooling as ONE 4-tap reduce over the (dh, dw) dims
                h2r = h2_t[:].rearrange(
                    "p (s hp dh wp dw) -> p (s hp) wp dh dw", s=NS, hp=3, dh=2, wp=3, dw=2
                )
                poolr = pool_t[0:6, :].rearrange("p (s hp wp) -> p (s hp) wp", s=NS, hp=3, wp=3)
                with nc.allow_low_precision("4-term bf16 pooling sum, matches prior impl"):
                    vector.tensor_reduce(
                        poolr[:], h2r[:], axis=mybir.AxisListType.XY, op=mybir.AluOpType.add
                    )._wait_ge(asem, 2).then_inc(vsem)  # 1


            @block.gpsimd
            def _(gpsimd):
                # w4p first on the Pool SWDGE (fc4 needs it earliest of the
                # SWDGE loads), then the memsets (the ACT dummy waits
                # ssem>=5), then the third w68 piece (moving it off the
                # 6-DMA Sync ring whose congested completion path stalled
                # fc6 ~1us). HWDGE and SWDGE must not share a semaphore.
                # Whole-tensor memsets: APs must start at partition 0.
                hoist.append(gpsimd.dma_start(out=wb2_t[:], in_=wb2_d[:]).then_inc(dsB, 16))
                hoist.append(gpsimd.dma_start(out=w4p_t[:], in_=w4p_d[:]).then_inc(dsE, 16))
                hoist.append(gpsimd.memset(h1_t[:], 1.0).then_inc(ssem))
                hoist.append(gpsimd.memset(pool_t[:], 1.0).then_inc(ssem))
                hoist.append(gpsimd.memset(h4_t[:], 1.0).then_inc(ssem))
                hoist.append(gpsimd.memset(t01[:], 1.0).then_inc(ssem))
                hoist.append(gpsimd.memset(zb_t[:], 0.0).then_inc(ssem))
                hoist.append(gpsimd.dma_start(out=w5t_t[:], in_=w5t_d[:]).then_inc(dsF, 16))

            @block.scalar
            def _(scalar):
                # Scalar stays DMA-free so its sequencer reaches the table
                # load ASAP. dummy sigmoid FIRST IN THE BB: walrus emits the
                # ACT_TABLE_LOAD right before the first table-using ACTIVATE
                # of the bb with no wait attached, so the ~1.3us table stream
                # overlaps the input-DMA completion wait. The dummy's own wait
                # covers the memsets.
                scalar.activation(scr_t[:], zb_t[0:1, 0:2], Sig, bias=zb_t[0:1, 0:1])._wait_ge(ssem, 5)
                scalar.activation(h1_t[0:24, :], psum1[0:24, :], Sig, bias=zb_t[0:24, 0:1])._wait_ge(psem, 1).then_inc(asem)  # 1
                scalar.activation(h2_t[:], psum2[:], Sig, bias=zb_t[0:6, 0:1])._wait_ge(psem, 2).then_inc(asem)  # 2
                scalar.activation(h4_t[0:80, :], psum4[:], Sig, bias=zb_t[0:80, 0:1])._wait_ge(psem, 3).then_inc(asem)  # 3
                # fc5 emits tanh(x/2) = 2*sigmoid(x)-1 in fp8 (fc6's
                # DoubleRow moving operand needs BOTH halves, so one ACT op)
                scalar.activation(
                    t01[0:100, :], psum5[:], Tanh, bias=zb_t[0:100, 0:1], scale=0.5
                )._wait_ge(psem, 4).then_inc(asem)  # 4
                scalar.activation(h6_t[:], psum6[0:52, :], Sig, bias=zb_t[0:52, 0:1])._wait_ge(psem, 5).then_inc(asem)  # 5
                scalar.copy(out_t[:, 0:168], psume[:, 0:168])._wait_ge(psem, 6).then_inc(asem)
                scalar.copy(out_t[:, 168:336], psume[:, 168:336])._wait_ge(psem, 7).then_inc(asem)

            @block.tensor
            def _(tensor):
                # p-state warm-up: a dep-free K=1 f32 matmul so the PE clock
                # ramps before conv1 (reads zb_t racily; output unused)
                tensor.matmul(psum_w[:], zb_t[0:1, 0:1], zb_t[0:1, 0:2], start=True, stop=True)
                # conv1: 4 fp8 DoubleRow matmuls, contraction 2x27 (52 x-rows
                # + split-b1 ones rows). The a8 gate rides the first matmul.
                a8w = a8_t[:, 0:256].rearrange("p (j t d) -> p j t d", j=4, t=2, d=32)
                a8x = a8_t[:, 256:1408].rearrange("p (j t n) -> p j t n", j=4, t=2, n=144)
                for j in range(4):
                    mm = tensor.matmul(
                        psum1[:],
                        a8w[:, j],
                        a8x[:, j],
                        start=(j == 0),
                        stop=(j == 3),
                        perf_mode=DR,
                    )
                    if j == 0:
                        mm._wait_ge(dsA, _DMA_CREDITS)
                    if j == 3:
                        mm.then_inc(psem)  # psem 1
                # conv2: K=25 incl. the b2 ones-row; stationary lives in wb2
                tensor.wait_ge(dsB, _DMA_CREDITS)
                h14 = h1_t[:].rearrange("p (s h w) -> p s h w", s=NS, h=6, w=12)
                for kw in range(2):
                    mm = tensor.matmul(
                        psum2[:],
                        wb2_t[0:25, kw * 6 : (kw + 1) * 6],
                        h14[:, :, :, kw : kw + 11 : 2],
                        start=(kw == 0),
                        stop=(kw == 1),
                    )
                    if kw == 0:
                        mm._wait_ge(asem, 1)
                    if kw == 1:
                        mm.then_inc(psem)  # psem 2
                tensor.wait_ge(dsE, _DMA_CREDITS)
                # fc4: 9 (hp,wp) matmuls vs the h/w-pooled tile; d-pooling and
                # /8 live in w4p; j=0 has K=7 incl. the b4 ones-row
                pool4 = pool_t[:].rearrange("p (s j) -> p s j", s=NS, j=9)
                for j in range(9):
                    kk = 7 if j == 0 else 6
                    mm = tensor.matmul(
                        psum4[:],
                        w4p_t[0:kk, j * 80 : (j + 1) * 80],
                        pool4[0:kk, :, j],
                        start=(j == 0),
                        stop=(j == 8),
                    )
                    if j == 0:
                        mm._wait_ge(vsem, 1)
                    if j == 8:
                        mm.then_inc(psem)  # psem 3
                # fc5: two k-halves into one psum (one tanh covers both)
                tensor.wait_ge(dsF, _DMA_CREDITS)
                tensor.matmul(
                    psum5[:, 0:NS], w5t_t[0:81, 0:100], h4_t[:], start=True, stop=True
                )._wait_ge(asem, 3)
                tensor.matmul(
                    psum5[:, NS : 2 * NS], w5t_t[0:81, 100:200], h4_t[:], start=True, stop=True
                ).then_inc(psem)  # psem 4
                # fc6: 13 fp8 DoubleRow matmuls, contraction 2x101 (200 tanh
                # rows + split-b6c ones rows); needs BOTH fc5 halves (asem>=5)
                tensor.wait_ge(dsG, 4 * _DMA_CREDITS)
                w68v = w68_t[:].rearrange("p (i t jj) -> p i t jj", i=13, t=2, jj=64)
                t01v = t01[:].rearrange("p (t s) -> p t s", t=2, s=NS)
                for i in range(13):
                    mm = tensor.matmul(
                        psum6[:, i * NS : (i + 1) * NS],
                        w68v[:, i],
                        t01v[:],
                        start=True,
                        stop=True,
                        perf_mode=DR,
                    )
                    if i == 0:
                        mm._wait_ge(asem, 4)
                    if i == 12:
                        mm.then_inc(psem)  # psem 5
                # einsum; each sample region releases its own copy engine
                tensor.wait_ge(dsX, _DMA_CREDITS)
                h6v = h6_t[:].rearrange("p (i s) -> p s i", s=NS)
                for s in range(NS):
                    mm = tensor.matmul(
                        psume[:, s * 168 : (s + 1) * 168],
                        h6v[:, s, :],
                        wx_t[0:52, s * 168 : (s + 1) * 168],
                        start=True,
                        stop=True,
                    ).then_inc(psem)  # psem 6, 7
                    if s == 0:
                        mm._wait_ge(asem, 5)

    _strip_barriers(nc)
    return nc


def _strip_barriers(nc):
    f = nc.m.functions[0]
    bbs = {bb.name: bb for bb in f.blocks}
    main = bbs["main"]
    # 1) drop the init all-engine barrier (nothing reads the const-AP tiles)
    main.instructions = [
        i
        for i in main.instructions
        if not (
            i.name.startswith("barrier_")
            or getattr(i, "opcode", "") == "Drain"
            or type(i).__name__ == "InstDrain"
        )
    ]
    # 2) drop the Block exit barrier (the runtime postamble begins with its
    #    own all-engine rendezvous); keep the Drains so the output DMA
    #    quiesces before the postamble's semaphore-file reset
    for bb in f.blocks:
        if bb.name.endswith("_end"):
            bb.instructions = [
                i for i in bb.instructions if type(i).__name__ != "InstEventSemaphore"
            ]
    # 3) hoist the input-DMA triggers + memsets into main so they start during
    #    the preamble, before the Block-entry rendezvous
    hoisted = {bi.ins.name for bi in getattr(nc, "_hoist_insts", [])}
    if hoisted:
        moved = []
        for bb in f.blocks:
            if bb.name == "main" or not bb.instructions:
                continue
            keep = []
            for i in bb.instructions:
                (moved if i.name in hoisted else keep).append(i)
            if len(keep) != len(bb.instructions):
                bb.instructions = keep
        # insert at the very top of main (after the entry Call): the DMA
        # triggers use only immediates + the parameter table, not the
        # preamble registers
        insts = main.instructions
        main.instructions = insts[:1] + moved + insts[1:]


def _prep_weights(w1, b1, w2, b2, w4, b4, w5, b5, w6, b6):
    f = np.float32
    w1v = np.asarray(w1, f)[0, 0]  # (6,2,2)
    w2v = np.asarray(w2, f)[0, 0, :, 0, :]  # (4,2)
    w4 = np.asarray(w4, f)
    w5 = np.asarray(w5, f)
    w6 = np.asarray(w6, f)
    b1 = np.asarray(b1, f)
    b2 = np.asarray(b2, f)
    b4 = np.asarray(b4, f)
    b5 = np.asarray(b5, f)
    b6 = np.asarray(b6, f)

    # conv1 stationary, fp8 DoubleRow: WW[c, j, d] over contraction c;
    # d padded 24->32 (DoubleRow M % 32 == 0), pad cols stay zero
    WW = np.zeros((54, 4, 32), f)
    for kd in range(6):
        for kh in range(2):
            for kw in range(2):
                for d in range(24):
                    WW[2 * d + kd, kh * 2 + kw, d] = w1v[kd, kh, kw]
    u1 = np.asarray(b1[0], FP8).astype(f)
    WW[52, 0, 0:24] = u1            # bias hi, tap 0 only
    WW[53, 0, 0:24] = b1[0] - u1    # bias residual (fp8 of the remainder)
    wb8 = np.transpose(WW.reshape(2, 27, 4, 32), (1, 2, 0, 3))  # (p, j, t, d)

    # conv2 stationary block (bf16)
    wb2 = np.zeros((25, 12), f)
    for kd in range(4):
        for kw in range(2):
            for d in range(6):
                wb2[4 * d + kd, kw * 6 + d] = w2v[kd, kw]
    wb2[24, 0:6] = b2[0]  # ones-row bias, kw=0 block only

    w4r = w4.reshape(80, 3, 3, 3) / 8.0
    w4q = np.transpose(w4r, (1, 2, 3, 0)).reshape(3, 720)
    w4p = np.zeros((12, 720), f)
    w4p[0:6:2, :] = w4q
    w4p[1:6:2, :] = w4q
    w4p[6, 0:80] = b4  # ones-row bias, j=0 block only

    w5t = np.zeros((86, 200), f)
    w5t[0:80, :] = w5.T
    w5t[80, :] = b5

    # fc6 stationary, fp8 DoubleRow over tanh-centered fc5 outputs:
    # h6pre = (w6/2) @ tanh + (b6 + rowsum(w6)/2); jj padded 52->64
    W6E = np.zeros((202, 13, 64), f)
    w6h = 0.5 * w6.T.reshape(200, 13, 52)  # (k, i, jj)
    W6E[0:100, :, 0:52] = w6h[0:100]
    W6E[101:201, :, 0:52] = w6h[100:200]
    b6c = b6 + 0.5 * w6.sum(axis=1)
    u6 = np.asarray(b6c, FP8).astype(f)
    W6E[100, :, 0:52] = u6.reshape(13, 52)                # (p=100, t=0): bias hi
    W6E[201, :, 0:52] = (b6c - u6).reshape(13, 52)        # (p=100, t=1): residual
    # (p, t) -> c = t*101 + p; cols = i*128 + t*64 + jj
    w68 = np.transpose(W6E.reshape(2, 101, 13, 64), (1, 2, 0, 3)).reshape(101, 1664)

    return dict(
        w4p=np.ascontiguousarray(w4p.astype(BF16)),
        w5t=np.ascontiguousarray(w5t.astype(BF16)),
        w68=np.ascontiguousarray(w68.astype(FP8)),
    ), wb8, wb2


def kernel(x, w1, b1, w2, b2, w4, b4, w5, b5, w6, b6, _trace=False):
    global LAST_EXEC_NS, LAST_RESULT
    from concourse.bass_utils import run_bass_kernel_spmd

    if "nc" not in _BUILT:
        _BUILT["nc"] = _build_nc()
    nc = _BUILT["nc"]

    xs = np.ascontiguousarray(np.asarray(x, np.float32).reshape(10, 52, 168))
    wd, wb8, wb2 = _prep_weights(w1, b1, w2, b2, w4, b4, w5, b5, w6, b6)

    in_maps = []
    for i in range(N_CORES):
        xc = np.transpose(np.stack([xs[a] for a in ASSIGN[i]]), (1, 0, 2))  # (52, NS, 168)


        # a8: conv1 fp8 pack. XS[c, j, s, h, w] over contraction c (54).
        x4 = xc.reshape(52, NS, 7, 24)
        XS = np.ones((54, 4, NS, 6, 12), np.float32)
        for kh in range(2):
            for kw in range(2):
                XS[0:52, kh * 2 + kw] = x4[:, :, kh : kh + 6, kw : kw + 23 : 2]
        x8 = np.transpose(XS.reshape(2, 27, 4, NS * 72), (1, 2, 0, 3))  # (p, j, t, n)

        ac = np.zeros((27, 1408), np.float32)
        ac[:, 0:256] = wb8.reshape(27, 256)
        ac[:, 256:1408] = x8.reshape(27, 1152)

        m = {
            "a8": np.ascontiguousarray(ac.astype(FP8)),
            "wb2": np.ascontiguousarray(wb2.astype(BF16)),
            "wx": np.ascontiguousarray(xc.reshape(52, NS * 168).astype(BF16)),
        }
        m.update(wd)
        in_maps.append(m)

    res = run_bass_kernel_spmd(nc, in_maps, core_ids=list(range(N_CORES)), trace=_trace)
    LAST_EXEC_NS = res.exec_time_ns
    LAST_RESULT = res

    out = np.zeros((10, 2184), np.float32)
    for i in range(N_CORES):
        o = res.results[i]["out"].astype(np.float32).reshape(13, NS, 168)
        for slot, b in enumerate(ASSIGN[i]):
            out[b] = o[:, slot, :].reshape(2184)
    return out
